# revision 55
# baseline (speedup 1.0000x reference)
"""Trainium2 Bass kernel for nn_Butterfly_1580547970089.

Butterfly multiply (n=1024, log_n=10, nstacks=nblocks=1) + bias over a
16384-row batch, data-parallel across 8 NeuronCores (2048 rows each).

Final design (v6) - every choice below was A/B-measured on hardware:
  * DRAM tensors are sub-major contiguous [4, 128, 4096] (4096 = 8 feature
    blocks x 512 batch cols). Input arrives as two 2MB slabs per chunk
    (half-chunk granularity), issued from the Pool engine (SWDGE) so no
    other engine queues behind the pacing waits. Outputs leave as two DMAs
    per 512-col sub (vect blocks 0.75MB + folded blocks 0.25MB) on SP.
    Per-DMA serialized cost on HW is ~0.85us, so DMA count is minimized
    subject to dependency granularity (coarser outputs measured WORSE:
    the single out slab waits on the whole chain).
  * Block order BLOCKS = vect tiles in P8-pair order, then the folded
    pair {4,6}. Stages 0-7 are dense 128x128 bf16 lhsT blocks
    (host-composed); FOLDED tiles also fold stage 8 (4 matmuls each),
    VECT tiles use 2. Stage-9 g_t and the stage-8 diagonal are baked into
    the weights; r9/s8 ratio coefficients are per-partition scalars.
  * PSUM [128,1024] tiles (2 banks) x4/sub; ACT evacuates at FD=1024
    (bf16 downcast), emission order ps0, ps3, ps1, ps2. (PSUM_W=2048
    benched faster but computes WRONG results - do not enable.)
  * Elementwise work at FD=512 (measured fastest per element; GPSIMD
    compute and big-FD ops both measured slower in context). DVE: s8
    tensor_scalar x6, z tt x6, v9 (2-scalar ts) x6, out tt x6 + folded
    tt, in data-ready order. ACT additionally runs the two u-sourced v9s
    as Identity activations (scale=r9, bias=b; zero cross-engine wait
    since the source is ACT's own evac) - DVE/ACT/PE balance at ~21us.
  * For_i with staggered_reset unrolls UNROLL=12 chunks per iteration;
    measured monotone improvement 4->8->12 (36.8us), flat at 16.
  * Engine-role separation matters: each engine's in-order sequencer
    carries exactly one kind of pacing wait (Pool=input issue, SP=output
    issue, ACT=evac, DVE=elementwise, PE=matmul). Mixing roles (e.g.
    outputs issued by ACT) head-of-line blocks and measured ~4us slower.

Correctness: rel err 5.1e-3 vs fp32 reference (bf16 I/O + weights).
HW exec time: ~33us/chunk-of-2048-rows steady state (baseline 44.3us).
"""
import numpy as np
import ml_dtypes

import concourse.mybir as mybir
import concourse.tile as tile
from concourse import bacc, bass_utils

F32 = mybir.dt.float32
BF16 = mybir.dt.bfloat16
MULT = mybir.AluOpType.mult
ADD = mybir.AluOpType.add

N_CORES = 8
BATCH = 16384
N = 1024
B_CORE = BATCH // N_CORES   # 2048
CHUNK = B_CORE
SUB = 512
N_SUBS = CHUNK // SUB       # 4

S7_PAIRS = [(0, 1), (2, 3), (4, 5), (6, 7)]
S8_PAIRS = [(0, 2), (1, 3), (4, 6), (5, 7)]
S9_PAIRS = [(0, 4), (1, 5), (2, 6), (3, 7)]
OTHER7 = {0: 1, 1: 0, 2: 3, 3: 2, 4: 5, 5: 4, 6: 7, 7: 6}
P8 = {0: 2, 2: 0, 1: 3, 3: 1, 4: 6, 6: 4, 5: 7, 7: 5}
P9 = {0: 4, 4: 0, 1: 5, 5: 1, 2: 6, 6: 2, 3: 7, 7: 3}

FOLDED = (4, 6)                    # stage 8 folded into PE (4 matmuls)
VECT = (0, 2, 1, 3, 5, 7)          # stage 8 on DVE (2 matmuls)
BLOCKS = VECT + FOLDED             # SBUF/DRAM block order
NV = len(VECT)
BLK = {t: i for i, t in enumerate(BLOCKS)}
SRC = {t: (t, OTHER7[t], P8[t], OTHER7[P8[t]]) for t in FOLDED}

_WOFF = {}
for _i, _t in enumerate(FOLDED):
    for _k in range(4):
        _WOFF[(_t, _k)] = (_i * 4 + _k) * 128
for _i, _t in enumerate(VECT):
    for _k in range(2):
        _WOFF[(_t, _k)] = (len(FOLDED) * 4 + _i * 2 + _k) * 128
N_WBLK = len(FOLDED) * 4 + len(VECT) * 2  # 20

PSUM_W = 1024                      # psum tile width (2 banks)
N_PS = 4096 // PSUM_W              # psum tiles per sub
UNROLL = 12                        # chunks per For_i iteration
STAGGERED = True                   # staggered-reset back edge vs full barrier
IN_POOL = True                     # issue input DMAs from Pool (SWDGE)
IN_FINE = False                    # one input DMA per sub (4/chunk) vs per-half
FINE_OUT = 2                       # 2=one DMA/sub, 1=two DMAs/sub, 0=per-half
V9_ACT01 = True                    # v9 of blocks 0,1 on ACT (Identity)
SKIP_IN = False                    # debug: skip input slab DMAs
SKIP_OUT = False                   # debug: skip output slab DMAs
STRIP = 0                          # debug: 2=mm only, 1=+evac, 4=+s8+v9, 0=full
USE_POOL = False                   # route folded-block v9/tt to Pool
FAKE_OUT = False                   # debug: out-DMAs read a constant tile
FAKE_BIG = False                   # debug: fake outs as 2 big DMAs per sub

_compiled = {}


def _emit_kernel(loop_reps=None):
    nc = bacc.Bacc("TRN2", target_bir_lowering=False, debug=False)
    xT = nc.dram_tensor("xT", [N_SUBS, 128, 4096], BF16,
                        kind="ExternalInput").ap()
    At = nc.dram_tensor("At", [128, N_WBLK * 128], BF16,
                        kind="ExternalInput").ap()
    coef = nc.dram_tensor("coef", [128, 32], F32, kind="ExternalInput").ap()
    outT = nc.dram_tensor("outT", [N_SUBS, 128, 4096], BF16,
                          kind="ExternalOutput").ap()

    with tile.TileContext(nc) as tc:
        with (
            tc.tile_pool(name="const", bufs=1) as cpool,
            tc.tile_pool(name="xin", bufs=4) as xpool,
            tc.tile_pool(name="uo", bufs=4) as upool,
            tc.tile_pool(name="v8o", bufs=3) as v8pool,
            tc.tile_pool(name="zo", bufs=3) as zpool,
            tc.tile_pool(name="v9o", bufs=4) as v9pool,
            tc.tile_pool(name="outb", bufs=2) as opool,
            tc.tile_pool(name="psum", bufs=1, space="PSUM") as ppool,
        ):
            at = cpool.tile([128, N_WBLK * 128], BF16, tag="at")
            nc.sync.dma_start(at[:], At[:])
            cf = cpool.tile([128, 32], F32, tag="cf")
            nc.sync.dma_start(cf[:], coef[:])

            def c(col):
                return cf[:, col:col + 1]

            def w(t, k):
                off = _WOFF[(t, k)]
                return at[:, off:off + 128]

            # ps emission order: ps0 first (s8 sources for vect pair 0),
            # ps3 second (folded blocks -> v9 sources), then ps1, ps2
            PS_ORDER = (0, 3, 1, 2) if N_PS == 4 else tuple(range(N_PS))

            def bslice(tile_, b):
                return tile_[:, b * 512:(b + 1) * 512]

            def body(in_loop=True):
                # input slabs: per-half 2MB (default) or per-sub 1MB
                ieng0 = nc.gpsimd if IN_POOL else nc.scalar
                xh = [None, None]
                if IN_FINE:
                    for h in range(2):
                        xh[h] = xpool.tile([128, 8192], BF16, tag="xs",
                                           name=f"xh{h}")
                        if not SKIP_IN:
                            for so_ in range(2):
                                s_ = 2 * h + so_
                                ieng0.dma_start(
                                    xh[h][:, so_ * 4096:(so_ + 1) * 4096],
                                    xT[s_])
                else:
                    for h in range(2):
                        xh[h] = xpool.tile([128, 8192], BF16, tag="xs",
                                           name=f"xh{h}")
                        if not SKIP_IN:
                            src3 = xT[2 * h:2 * h + 2].transpose([1, 0, 2])
                            ieng0.dma_start(xh[h][:, :], src3)
                oh = [None, None]

                for s in range(N_SUBS):
                    h, so = divmod(s, 2)
                    xoff = so * 4096

                    def xb(t):
                        b = BLK[t]
                        return xh[h][:, xoff + b * 512:xoff + (b + 1) * 512]

                    ps = [None] * N_PS
                    for i in range(N_PS):
                        ps[i] = ppool.tile([128, PSUM_W], F32, tag=f"ps{i}",
                                           name=f"ps{i}")
                    for i in PS_ORDER:
                        for b in (2 * i, 2 * i + 1):
                            t = BLOCKS[b]
                            col = (b * 512) % PSUM_W
                            out_ap = ps[i][:, col:col + 512]
                            if t in FOLDED:
                                for k in range(4):
                                    nc.tensor.matmul(out_ap, w(t, k),
                                                     xb(SRC[t][k]),
                                                     start=(k == 0),
                                                     stop=(k == 3))
                            else:
                                nc.tensor.matmul(out_ap, w(t, 0), xb(t),
                                                 start=True, stop=False)
                                nc.tensor.matmul(out_ap, w(t, 1),
                                                 xb(OTHER7[t]),
                                                 start=False, stop=True)

                    # --- evac: ACT, FD=PSUM_W, bf16 downcast ---
                    u = upool.tile([128, 4096], BF16, tag="u", name="u")
                    if STRIP == 2:
                        continue
                    for i in PS_ORDER:
                        nc.scalar.copy(u[:, i * PSUM_W:(i + 1) * PSUM_W],
                                       ps[i][:])

                    if STRIP == 1:
                        continue
                    if STRIP == 5:
                        if so == 0:
                            oh[h] = opool.tile([128, 8192], BF16, tag="ot",
                                               name=f"oh{h}")
                        nc.vector.tensor_copy(oh[h][:, xoff:xoff + 4096],
                                              u[:, :])
                        if so == 1 and not SKIP_OUT:
                            dst3 = outT[2 * h:2 * h + 2].transpose([1, 0, 2])
                            nc.sync.dma_start(dst3, oh[h][:, :])
                        continue
                    v8 = v8pool.tile([128, NV * 512], BF16, tag="v8",
                                     name="v8")
                    z = zpool.tile([128, NV * 512], BF16, tag="z", name="z")
                    v9 = v9pool.tile([128, 4096], BF16, tag="v9", name="v9")
                    if so == 0:
                        oh[h] = opool.tile([128, 8192], BF16, tag="ot",
                                           name=f"oh{h}")
                    ot = oh[h][:, xoff:xoff + 4096]

                    def v8_op(i):
                        t = VECT[i]
                        nc.vector.tensor_scalar(
                            bslice(v8, i), bslice(u, BLK[P8[t]]),
                            c(16 + i), None, op0=MULT)

                    def z_op(i):
                        nc.vector.tensor_tensor(bslice(z, i), bslice(u, i),
                                                bslice(v8, i), op=ADD)

                    def v9_src(b):
                        pb = BLK[P9[BLOCKS[b]]]
                        return bslice(z, pb) if pb < NV else bslice(u, pb)

                    def v9_op(b, eng):
                        t = BLOCKS[b]
                        if eng is nc.scalar:
                            nc.scalar.activation(
                                bslice(v9, b), v9_src(b),
                                mybir.ActivationFunctionType.Identity,
                                bias=c(8 + t), scale=c(t))
                        else:
                            eng.tensor_scalar(bslice(v9, b), v9_src(b),
                                              c(t), c(8 + t),
                                              op0=MULT, op1=ADD)

                    def out_op(b):
                        nc.vector.tensor_tensor(bslice(ot, b), bslice(z, b),
                                                bslice(v9, b), op=ADD)

                    # DVE emission in data-ready order
                    v8_op(0); v8_op(1)
                    z_op(0); z_op(1)
                    e01 = nc.scalar if V9_ACT01 else nc.vector
                    v9_op(0, e01); v9_op(1, e01)  # src u b6,b7
                    if STRIP != 4:
                        out_op(0); out_op(1)
                    # folded-block v9 on DVE (sources z b0,b1 just made);
                    # Pool gets only the terminal folded out-tt
                    peng = nc.gpsimd if USE_POOL else nc.vector
                    if STRIP != 4:
                        v9_op(6, nc.vector)   # src z b0
                        v9_op(7, nc.vector)   # src z b1
                        peng.tensor_tensor(ot[:, NV * 512:],
                                           u[:, NV * 512:],
                                           v9[:, NV * 512:], op=ADD)
                    # DVE continues
                    v8_op(2); v8_op(3)
                    z_op(2); z_op(3)
                    v8_op(4); v8_op(5)
                    z_op(4); z_op(5)
                    v9_op(4, nc.vector); v9_op(5, nc.vector)  # src z b2,b3
                    v9_op(2, nc.vector); v9_op(3, nc.vector)  # src z b4,b5
                    if STRIP == 4:
                        continue
                    out_op(2); out_op(3); out_op(4); out_op(5)

                    # --- output DMAs ---
                    if not SKIP_OUT:
                        if FINE_OUT == 2:
                            nc.sync.dma_start(outT[s][:, :], ot[:, :])
                        elif FINE_OUT:
                            nc.sync.dma_start(outT[s][:, 0:NV * 512],
                                              ot[:, 0:NV * 512])
                            nc.sync.dma_start(outT[s][:, NV * 512:],
                                              ot[:, NV * 512:])
                        elif so == 1:
                            dst3 = outT[2 * h:2 * h + 2].transpose([1, 0, 2])
                            nc.sync.dma_start(dst3, oh[h][:, :])

            if loop_reps is not None:
                with tc.For_i(0, max(loop_reps // UNROLL, 1), 1,
                              staggered_reset=STAGGERED,
                              hint_engines=(mybir.EngineType.PE,
                                            mybir.EngineType.DVE,
                                            mybir.EngineType.Activation)):
                    for _ in range(UNROLL):
                        body()
            else:
                body(in_loop=False)

    nc.compile()
    return nc


def _get_compiled(loop_reps=None):
    if loop_reps not in _compiled:
        _compiled[loop_reps] = _emit_kernel(loop_reps)
    return _compiled[loop_reps]


def _build_A(twiddle):
    A = np.zeros((8, 128, 128), np.float64)
    for h in range(8):
        M = np.eye(128, dtype=np.float64)
        for idx in range(7):
            s = 1 << idx
            tw = twiddle[0, 0, idx].astype(np.float64).reshape(512 // s, s, 2, 2)
            tw_h = tw[h * (64 // s):(h + 1) * (64 // s)]
            Mv = M.reshape(64 // s, 2, s, 128)
            top, bot = Mv[:, 0], Mv[:, 1]
            M = np.stack(
                [tw_h[:, :, 0, 0][..., None] * top + tw_h[:, :, 0, 1][..., None] * bot,
                 tw_h[:, :, 1, 0][..., None] * top + tw_h[:, :, 1, 1][..., None] * bot],
                axis=1).reshape(128, 128)
        A[h] = M
    return A


def _coef_parts(twiddle):
    t8 = twiddle[0, 0, 8].reshape(2, 256, 2, 2).astype(np.float64)
    t9 = twiddle[0, 0, 9].reshape(512, 2, 2).astype(np.float64)
    c8d = np.zeros((8, 128)); c8o = np.zeros((8, 128))
    for gi, (p_, q_) in enumerate(S8_PAIRS):
        G, hp = divmod(gi, 2)
        cc = t8[G, hp * 128:(hp + 1) * 128]
        c8d[p_], c8o[p_] = cc[:, 0, 0], cc[:, 0, 1]
        c8d[q_], c8o[q_] = cc[:, 1, 1], cc[:, 1, 0]
    g = np.zeros((8, 128)); r9 = np.zeros((8, 128))
    for a, b in S9_PAIRS:
        e = t9[a * 128:(a + 1) * 128]
        g[a], g[b] = e[:, 0, 0], e[:, 1, 1]
        r9[a] = e[:, 0, 1] / e[:, 1, 1]
        r9[b] = e[:, 1, 0] / e[:, 0, 0]
    return c8d, c8o, g, r9


def _build_weights(twiddle):
    """At [128, N_WBLK*128] bf16 per _WOFF layout (lhsT blocks)."""
    A = _build_A(twiddle)
    t7 = twiddle[0, 0, 7].reshape(4, 128, 2, 2).astype(np.float64)
    B = np.zeros((8, 128, 128)); C = np.zeros((8, 128, 128))
    for gi, (p, q) in enumerate(S7_PAIRS):
        B[p] = np.diag(t7[gi, :, 0, 0]) @ A[p]
        C[p] = np.diag(t7[gi, :, 0, 1]) @ A[q]
        B[q] = np.diag(t7[gi, :, 1, 1]) @ A[q]
        C[q] = np.diag(t7[gi, :, 1, 0]) @ A[p]
    c8d, c8o, g, _ = _coef_parts(twiddle)
    At = np.zeros((128, N_WBLK * 128), ml_dtypes.bfloat16)
    for t in FOLDED:
        pt = P8[t]
        Ws = [
            (g[t] * c8d[t])[:, None] * B[t],
            (g[t] * c8d[t])[:, None] * C[t],
            (g[t] * c8o[t])[:, None] * B[pt],
            (g[t] * c8o[t])[:, None] * C[pt],
        ]
        for k, W in enumerate(Ws):
            off = _WOFF[(t, k)]
            At[:, off:off + 128] = W.T.astype(ml_dtypes.bfloat16)
    for t in VECT:
        alpha = g[t] * c8d[t]
        for k, W in enumerate((alpha[:, None] * B[t], alpha[:, None] * C[t])):
            off = _WOFF[(t, k)]
            At[:, off:off + 128] = W.T.astype(ml_dtypes.bfloat16)
    return At


def _build_coef(twiddle, bias):
    c8d, c8o, g, r9 = _coef_parts(twiddle)
    coef = np.zeros((128, 32), np.float32)
    coef[:, 0:8] = r9.T
    coef[:, 8:16] = np.asarray(bias, np.float64).reshape(8, 128).T
    for i, t in enumerate(VECT):
        alpha_p = g[P8[t]] * c8d[P8[t]]
        coef[:, 16 + i] = g[t] * c8o[t] / alpha_p
    return coef


def _build_xT(shard):
    """shard [B_CORE, 1024] fp32 -> [4, 128, 4096] bf16, sub-major."""
    xt = np.ascontiguousarray(shard.T).astype(ml_dtypes.bfloat16)
    xtb = xt.reshape(8, 128, -1)[list(BLOCKS)]      # [8,128,2048] block-major
    n_subs = xtb.shape[2] // SUB
    out = xtb.reshape(8, 128, n_subs, SUB).transpose(2, 1, 0, 3)
    return np.ascontiguousarray(out.reshape(n_subs, 128, 8 * SUB))


def kernel(input, twiddle, bias):
    input = np.asarray(input)
    twiddle = np.asarray(twiddle)
    bias = np.asarray(bias)
    nc = _get_compiled()

    At = _build_weights(twiddle)
    coef = _build_coef(twiddle, bias)
    in_maps = []
    for cid in range(N_CORES):
        shard = input[cid * B_CORE:(cid + 1) * B_CORE, :]
        in_maps.append({"xT": _build_xT(shard), "At": At, "coef": coef})

    res = bass_utils.run_bass_kernel_spmd(nc, in_maps,
                                          core_ids=list(range(N_CORES)))
    inv = [BLK[t] for t in range(8)]  # tile t lives at block BLK[t]
    out = np.empty((BATCH, N), np.float32)
    for cid in range(N_CORES):
        o = np.asarray(res.results[cid]["outT"])    # [4, 128, 4096] bf16
        ob = o.reshape(N_SUBS, 128, 8, SUB).transpose(2, 1, 0, 3)
        full = ob.reshape(8, 128, CHUNK)[inv].reshape(N, CHUNK)
        out[cid * B_CORE:(cid + 1) * B_CORE, :] = full.T.astype(np.float32)
    return out
